# revision 26
# baseline (speedup 1.0000x reference)
"""
GeneNetworkGreensFunction kernel for 8 Trainium2 NeuronCores.

Math (Woodbury): with z = omega + i*eta, D = z*I - diag(d) (diagonal),
H = U U^T + diag(d):
    R = z*I - H = D - U U^T
    G = R^{-1} = D^{-1} + W M W^T,  W = D^{-1} U,  M = (I_r - U^T D^{-1} U)^{-1}
Output = min(|G|, 10) as float32, shape [n, n].

The small algebra (a = 1/(z-d), W [n,32], V = W M) runs on host in
complex128.  The O(n^2 r) dense part — the complex outer product V W^T and
its magnitude — runs on the 8 NeuronCores.

G is symmetric (M is complex-symmetric), so only the upper block triangle
is computed: the 36 unordered pairs of 512-row blocks are distributed
rotationally — core c computes pairs {c,c}, {c,c+1}, {c,c+2}, {c,c+3}
(mod 8) plus half of {c%4, c%4+4} — 4.5 block-pairs (18 [128,512] tiles)
per core.  The host mirrors the strict-lower blocks and patches the exact
diagonal (which also removes the need for any on-device clamp: max
off-diagonal |G| ~ 0.3 << 10).

Complex matmul is packed as two real matmuls with K = 2*rank = 64 in bf16
(fp32 matmul streams at 1/4 rate; bf16 end-to-end rel err ~4e-4 vs the
2e-2 gate).  The 're' matmuls use PE rows 0-63 and the 'im' matmuls rows
64-127 (tile_position row packing) so they run concurrently, filling a
4-bank [128, 2048] PSUM supertile (re | im).  Epilogue, balanced so ACT
and DVE busy-time match (an instruction may read only one PSUM operand,
so DVE must copy before it can square): DVE casts cols [0:XD) to bf16 and
squares them 2x-packed; ACT Squares cols [XD:2048) straight from PSUM;
supertiles are processed in pairs with one gathered DVE bf16 add and one
2048-wide ACT sqrt -> f32 -> DMA.  HW: ~39 us on 8 cores (baseline 164).
"""

import sys

for _p in ("/opt/trn_rl_repo",):
    if _p not in sys.path:
        sys.path.insert(0, _p)

import numpy as np

N = 4096
RANK = 32
CLAMP = 10.0
NCORES = 8
BLK = 512                    # block size (N / NCORES)
MT = 128                     # output tile partition rows
NT = 512                     # matmul free columns (one PSUM bank)
ST = 2 * NT                  # supertile free width (two PSUM banks)
K2 = 2 * RANK                # packed contraction dim
NSUPER = 9                   # supertiles per core
XD = 824                     # columns (of 2048) squared on DVE per supertile

# consts layout (columns, bf16):
#   [slot0 slot1 | rhsblk0 rhsblk1 | slot2..slot5 | rhsblk2 rhsblk3 rhsblk4]
# so the first DMA chunk (cols 0:1280) covers everything the first
# supertile pair needs.
LHS_SLOTS = 6                # 4 own-mc slots + 2 half-block slots
RHS_BLOCKS = 5               # col blocks c, c+1, c+2, c+3, (c%4)+4
SLOT_COL = [0, 128, 1280, 1408, 1536, 1664]
RHSB_COL = [256, 768, 1792, 2304, 2816]
CONSTS_W = 3328
CHUNKS = [(0, 768), (768, 1280), (1280, 3328)]

# supertile schedule: (lhs_slot_a, lhs_slot_b, rhs_blk_a, rhs_blk_b), paired
# so each pair shares its rhs blocks: (mc0,mc1)x(blk0,blk1), (mc2,mc3)x..
SCHED = [
    (0, 0, 0, 1), (1, 1, 0, 1),
    (2, 2, 0, 1), (3, 3, 0, 1),
    (0, 0, 2, 3), (1, 1, 2, 3),
    (2, 2, 2, 3), (3, 3, 2, 3),
    (4, 5, 4, 4),
]

_CACHE = {}


def _build_program():
    import concourse.bass as bass
    import concourse.mybir as mybir
    import concourse.tile as tile
    from concourse import bacc

    f32 = mybir.dt.float32
    bf16 = mybir.dt.bfloat16
    nc = bacc.Bacc(
        "TRN2", target_bir_lowering=False, debug=False, num_devices=NCORES
    )

    consts = nc.declare_dram_parameter("consts", [128, CONSTS_W], bf16, isOutput=False)
    out = nc.declare_dram_parameter("out", [NSUPER * MT, ST], f32, isOutput=True)

    sched = SCHED

    # PSUM rule: an instruction may read only ONE non-scalar PSUM operand,
    # so each supertile's squares are column-split: ACT Squares cols
    # [XD:2048) straight from PSUM; DVE extracts cols [0:XD) (cast to bf16,
    # then one packed-bf16 multiply covering the whole pair).  Supertiles
    # run in pairs so the gathered add and the sqrt go 2048 wide,
    # amortizing per-instruction and semaphore overhead.

    with tile.TileContext(nc) as tc:
        with (
            tc.tile_pool(name="consts", bufs=1) as cpool,
            tc.tile_pool(name="ps", bufs=2, space="PSUM") as ps_pool,
            tc.tile_pool(name="sq", bufs=4) as sq_pool,
            tc.tile_pool(name="tcopy", bufs=4) as tcopy_pool,
            tc.tile_pool(name="ssum", bufs=4) as ssum_pool,
            tc.tile_pool(name="outp", bufs=4) as opool,
            tc.tile_pool(name="warm", bufs=1) as warm_pool,
        ):
            t_c = cpool.tile([128, CONSTS_W], bf16, tag="consts")
            # first-pair dependencies up front, rest behind.  Chunk 0 issues
            # from the ACT engine's HWDGE ring (alive before the sync engine
            # finishes its preamble) so the first matmul's data lands sooner;
            # the remaining chunks stream in parallel on the sync ring.
            for ci, (c0, c1) in enumerate(CHUNKS):
                eng = nc.scalar if ci == 0 else nc.sync
                eng.dma_start(
                    out=t_c[:, bass.ds(c0, c1 - c0)], in_=consts[:, bass.ds(c0, c1 - c0)]
                )

            # Warm the ACT function tables (Square/Sqrt load ~1.5us each)
            # while the consts DMA streams in.
            w_t = warm_pool.tile([128, 8], f32, tag="warm")
            nc.vector.memset(w_t[:], 0.0)
            nc.scalar.square(w_t[:, 0:4], w_t[:, 4:8])
            nc.scalar.sqrt(w_t[:, 0:4], w_t[:, 4:8])


            n_pairs = (NSUPER + 1) // 2
            for p in range(n_pairs):
                members = [2 * p] if 2 * p + 1 >= NSUPER else [2 * p, 2 * p + 1]
                nm = len(members)
                # sq holds both supertiles: [re2_a | im2_a | re2_b | im2_b]
                sq = sq_pool.tile([MT, nm * 2 * ST], bf16, tag=f"sq{nm}")
                for k, s in enumerate(members):
                    sa, sb, ca, cb = sched[s]
                    ps = ps_pool.tile([MT, 2 * ST], f32, tag="ps")
                    for half, (slot, cblk) in enumerate(((sa, ca), (sb, cb))):
                        l_re = t_c[0:K2, bass.ds(SLOT_COL[slot], MT)]
                        l_im = t_c[K2:128, bass.ds(SLOT_COL[slot], MT)]
                        r_re = t_c[0:K2, bass.ds(RHSB_COL[cblk], NT)]
                        r_im = t_c[K2:128, bass.ds(RHSB_COL[cblk], NT)]
                        nc.tensor.matmul(
                            ps[:, bass.ds(half * NT, NT)], l_re, r_re,
                            start=True, stop=True, tile_position=(0, 0),
                        )
                        nc.tensor.matmul(
                            ps[:, bass.ds(ST + half * NT, NT)], l_im, r_im,
                            start=True, stop=True, tile_position=(64, 0),
                        )
                    # column-split squares for this supertile
                    base = k * 2 * ST
                    nc.scalar.square(
                        sq[:, bass.ds(base + XD, 2 * ST - XD)],
                        ps[:, bass.ds(XD, 2 * ST - XD)],
                    )                                                   # ACT
                    if k == 0:
                        t_cp = tcopy_pool.tile([MT, nm * XD], bf16, tag=f"tcp{nm}")
                    nc.vector.tensor_copy(
                        t_cp[:, bass.ds(k * XD, XD)], ps[:, bass.ds(0, XD)]
                    )                                                   # DVE
                if True:
                    # one packed-bf16 square covering every member's DVE columns
                    sqv = sq[:].rearrange("p (s c) -> p s c", s=2 * nm)
                    nc.vector.tensor_mul(
                        sqv[:, 0:2 * nm:2, 0:XD],
                        t_cp[:].rearrange("p (s c) -> p s c", s=nm),
                        t_cp[:].rearrange("p (s c) -> p s c", s=nm),
                    )                                                   # DVE 2x 2x
                # fused add across the pair: gather [re2_a, re2_b] + [im2_a, im2_b]
                s_t = ssum_pool.tile([MT, nm * ST], bf16, tag=f"ssum{nm}")
                sq3 = sq[:].rearrange("p (s c) -> p s c", s=2 * nm)
                nc.vector.tensor_add(
                    s_t[:].rearrange("p (s c) -> p s c", s=nm),
                    sq3[:, 0:2 * nm:2, :],
                    sq3[:, 1:2 * nm:2, :],
                )                                                       # DVE 2x
                o = opool.tile([MT, nm * ST], f32, tag=f"o{nm}")
                if nm > 1:
                    nc.scalar.sqrt(o[:], s_t[:])                        # ACT
                    nc.sync.dma_start(
                        out=out[bass.ds(members[0] * MT, nm * MT), :]
                            .rearrange("(s p) c -> p s c", s=nm),
                        in_=o[:].rearrange("p (s c) -> p s c", s=nm),
                    )
                else:
                    # final supertile: halve the sqrt so the first half's DMA
                    # overlaps the second sqrt and the kernel-ending transfer
                    # (which exec time waits on) is only 256KB
                    for h in range(2):
                        nc.scalar.sqrt(
                            o[:, bass.ds(h * NT, NT)], s_t[:, bass.ds(h * NT, NT)]
                        )
                        nc.sync.dma_start(
                            out=out[bass.ts(members[0], MT), bass.ds(h * NT, NT)],
                            in_=o[:, bass.ds(h * NT, NT)],
                        )
    nc.finalize()
    return nc


def _woodbury_host(omega, U, d, log_eta):
    """complex128 host algebra. Returns a [n], V [n,r], W [n,r]."""
    U = np.asarray(U, np.float64)
    d = np.asarray(d, np.float64)
    eta = float(np.exp(np.float64(np.asarray(log_eta))))
    z = complex(float(np.asarray(omega)), eta)
    a = 1.0 / (z - d)                      # [n] complex128
    W = a[:, None] * U                     # [n, r]
    B = U.T @ W                            # [r, r]
    M = np.linalg.inv(np.eye(RANK) - B)    # [r, r]
    V = W @ M                              # [n, r]
    return a, V, W


def _core_layout(c):
    """(lhs row slices, rhs col blocks) for core c."""
    # lhs slots 0-3: mc tiles of row block c; slots 4-5: half-block rows
    hb = c % 4
    lhs_rows = [c * BLK + m * MT for m in range(4)]
    off = 0 if c < 4 else 2
    lhs_rows += [hb * BLK + (off + m) * MT for m in range(2)]
    rhs_blocks = [(c + d) % 8 for d in range(4)] + [hb + 4]
    return lhs_rows, rhs_blocks


def _prepare(omega, H_low_rank, H_diag, log_eta):
    """Host Woodbury + per-core input maps. Returns (in_maps, diag_vals)."""
    import ml_dtypes

    a, V, W = _woodbury_host(omega, H_low_rank, H_diag, log_eta)
    Vr = V.real.astype(np.float32); Vi = V.imag.astype(np.float32)
    Wr = W.real.astype(np.float32); Wi = W.imag.astype(np.float32)

    in_maps = []
    for c in range(NCORES):
        consts = np.zeros((128, CONSTS_W), np.float32)
        lhs_rows, rhs_blocks = _core_layout(c)
        for m, r0 in enumerate(lhs_rows):
            cs = slice(SLOT_COL[m], SLOT_COL[m] + MT)
            consts[0:RANK, cs] = Vr[r0:r0 + MT].T
            consts[RANK:K2, cs] = -Vi[r0:r0 + MT].T
            consts[K2:K2 + RANK, cs] = Vr[r0:r0 + MT].T
            consts[K2 + RANK:128, cs] = Vi[r0:r0 + MT].T
        for j, g in enumerate(rhs_blocks):
            cs = slice(RHSB_COL[j], RHSB_COL[j] + NT)
            g0 = g * BLK
            consts[0:RANK, cs] = Wr[g0:g0 + NT].T
            consts[RANK:K2, cs] = Wi[g0:g0 + NT].T
            consts[K2:K2 + RANK, cs] = Wi[g0:g0 + NT].T
            consts[K2 + RANK:128, cs] = Wr[g0:g0 + NT].T
        in_maps.append({"consts": consts.astype(ml_dtypes.bfloat16)})

    diag = a + np.einsum("ij,ij->i", V, W)             # G[i,i] = a_i + (V W^T)[i,i]
    return in_maps, np.minimum(np.abs(diag), CLAMP).astype(np.float32)


def _assemble(results, diag_vals):
    out = np.empty((N, N), np.float32)
    mirrored = []
    for c in range(NCORES):
        R = results[c]["out"]                          # [1152, 1024]
        lhs_rows, rhs_blocks = _core_layout(c)
        for s in range(8):
            mc = SCHED[s][0]
            r0 = c * BLK + mc * MT
            for half in (0, 1):
                g = rhs_blocks[SCHED[s][2 + half]]
                out[r0:r0 + MT, g * BLK:g * BLK + NT] = \
                    R[s * MT:(s + 1) * MT, half * NT:(half + 1) * NT]
        g = rhs_blocks[4]
        for half in (0, 1):
            r0 = lhs_rows[4 + half]
            out[r0:r0 + MT, g * BLK:g * BLK + NT] = \
                R[8 * MT:9 * MT, half * NT:(half + 1) * NT]
        # strict-upper pairs this core computed (for mirroring)
        for d in range(1, 4):
            mirrored.append((c, (c + d) % 8))
        if c < 4:
            mirrored.append((c, c + 4))
    for (i, j) in mirrored:
        out[j * BLK:(j + 1) * BLK, i * BLK:(i + 1) * BLK] = \
            out[i * BLK:(i + 1) * BLK, j * BLK:(j + 1) * BLK].T
    np.fill_diagonal(out, diag_vals)
    return out


def kernel(omega, H_low_rank, H_diag, log_eta):
    from concourse.bass_utils import run_bass_kernel_spmd

    in_maps, diag_vals = _prepare(omega, H_low_rank, H_diag, log_eta)
    if "nc" not in _CACHE:
        _CACHE["nc"] = _build_program()
    res = run_bass_kernel_spmd(_CACHE["nc"], in_maps, list(range(NCORES)))
    return _assemble(res.results, diag_vals)


# revision 27
# speedup vs baseline: 1.0042x; 1.0042x over previous
"""
GeneNetworkGreensFunction kernel for 8 Trainium2 NeuronCores.

Math (Woodbury): with z = omega + i*eta, D = z*I - diag(d) (diagonal),
H = U U^T + diag(d):
    R = z*I - H = D - U U^T
    G = R^{-1} = D^{-1} + W M W^T,  W = D^{-1} U,  M = (I_r - U^T D^{-1} U)^{-1}
Output = min(|G|, 10) as float32, shape [n, n].

The small algebra (a = 1/(z-d), W [n,32], V = W M) runs on host in
complex128.  The O(n^2 r) dense part — the complex outer product V W^T and
its magnitude — runs on the 8 NeuronCores.

G is symmetric (M is complex-symmetric), so only the upper block triangle
is computed: the 36 unordered pairs of 512-row blocks are distributed
rotationally — core c computes pairs {c,c}, {c,c+1}, {c,c+2}, {c,c+3}
(mod 8) plus half of {c%4, c%4+4} — 4.5 block-pairs (18 [128,512] tiles)
per core.  The host mirrors the strict-lower blocks and patches the exact
diagonal (which also removes the need for any on-device clamp: max
off-diagonal |G| ~ 0.3 << 10).

Complex matmul is packed as two real matmuls with K = 2*rank = 64 in bf16
(fp32 matmul streams at 1/4 rate; bf16 end-to-end rel err ~4e-4 vs the
2e-2 gate).  The 're' matmuls use PE rows 0-63 and the 'im' matmuls rows
64-127 (tile_position row packing) so they run concurrently, filling a
4-bank [128, 2048] PSUM supertile (re | im).  Epilogue, balanced so ACT
and DVE busy-time match (an instruction may read only one PSUM operand,
so DVE must copy before it can square): DVE casts cols [0:XD) to bf16 and
squares them 2x-packed; ACT Squares cols [XD:2048) straight from PSUM;
supertiles are processed in pairs with one gathered DVE bf16 add and one
2048-wide ACT sqrt -> f32 -> DMA.  HW: ~39 us on 8 cores (baseline 164).
"""

import sys

for _p in ("/opt/trn_rl_repo",):
    if _p not in sys.path:
        sys.path.insert(0, _p)

import numpy as np

N = 4096
RANK = 32
CLAMP = 10.0
NCORES = 8
BLK = 512                    # block size (N / NCORES)
MT = 128                     # output tile partition rows
NT = 512                     # matmul free columns (one PSUM bank)
ST = 2 * NT                  # supertile free width (two PSUM banks)
K2 = 2 * RANK                # packed contraction dim
NSUPER = 9                   # supertiles per core
XD = 800                     # columns (of 2048) squared on DVE per supertile

# consts layout (columns, bf16):
#   [slot0 slot1 | rhsblk0 rhsblk1 | slot2..slot5 | rhsblk2 rhsblk3 rhsblk4]
# so the first DMA chunk (cols 0:1280) covers everything the first
# supertile pair needs.
LHS_SLOTS = 6                # 4 own-mc slots + 2 half-block slots
RHS_BLOCKS = 5               # col blocks c, c+1, c+2, c+3, (c%4)+4
SLOT_COL = [0, 128, 1280, 1408, 1536, 1664]
RHSB_COL = [256, 768, 1792, 2304, 2816]
CONSTS_W = 3328
CHUNKS = [(0, 768), (768, 1280), (1280, 3328)]

# supertile schedule: (lhs_slot_a, lhs_slot_b, rhs_blk_a, rhs_blk_b), paired
# so each pair shares its rhs blocks: (mc0,mc1)x(blk0,blk1), (mc2,mc3)x..
SCHED = [
    (0, 0, 0, 1), (1, 1, 0, 1),
    (2, 2, 0, 1), (3, 3, 0, 1),
    (0, 0, 2, 3), (1, 1, 2, 3),
    (2, 2, 2, 3), (3, 3, 2, 3),
    (4, 5, 4, 4),
]

_CACHE = {}


def _build_program():
    import concourse.bass as bass
    import concourse.mybir as mybir
    import concourse.tile as tile
    from concourse import bacc

    f32 = mybir.dt.float32
    bf16 = mybir.dt.bfloat16
    nc = bacc.Bacc(
        "TRN2", target_bir_lowering=False, debug=False, num_devices=NCORES
    )

    consts = nc.declare_dram_parameter("consts", [128, CONSTS_W], bf16, isOutput=False)
    out = nc.declare_dram_parameter("out", [NSUPER * MT, ST], f32, isOutput=True)

    sched = SCHED

    # PSUM rule: an instruction may read only ONE non-scalar PSUM operand,
    # so each supertile's squares are column-split: ACT Squares cols
    # [XD:2048) straight from PSUM; DVE extracts cols [0:XD) (cast to bf16,
    # then one packed-bf16 multiply covering the whole pair).  Supertiles
    # run in pairs so the gathered add and the sqrt go 2048 wide,
    # amortizing per-instruction and semaphore overhead.

    with tile.TileContext(nc) as tc:
        with (
            tc.tile_pool(name="consts", bufs=1) as cpool,
            tc.tile_pool(name="ps", bufs=2, space="PSUM") as ps_pool,
            tc.tile_pool(name="sq", bufs=3) as sq_pool,
            tc.tile_pool(name="tcopy", bufs=3) as tcopy_pool,
            tc.tile_pool(name="ssum", bufs=3) as ssum_pool,
            tc.tile_pool(name="outp", bufs=3) as opool,
            tc.tile_pool(name="warm", bufs=1) as warm_pool,
        ):
            t_c = cpool.tile([128, CONSTS_W], bf16, tag="consts")
            # first-pair dependencies up front, rest behind.  Chunk 0 issues
            # from the ACT engine's HWDGE ring (alive before the sync engine
            # finishes its preamble) so the first matmul's data lands sooner;
            # the remaining chunks stream in parallel on the sync ring.
            for ci, (c0, c1) in enumerate(CHUNKS):
                eng = nc.scalar if ci == 0 else nc.sync
                eng.dma_start(
                    out=t_c[:, bass.ds(c0, c1 - c0)], in_=consts[:, bass.ds(c0, c1 - c0)]
                )

            # Warm the ACT function tables (Square/Sqrt load ~1.5us each)
            # while the consts DMA streams in.
            w_t = warm_pool.tile([128, 8], f32, tag="warm")
            nc.vector.memset(w_t[:], 0.0)
            nc.scalar.square(w_t[:, 0:4], w_t[:, 4:8])
            nc.scalar.sqrt(w_t[:, 0:4], w_t[:, 4:8])


            n_pairs = (NSUPER + 1) // 2
            for p in range(n_pairs):
                members = [2 * p] if 2 * p + 1 >= NSUPER else [2 * p, 2 * p + 1]
                nm = len(members)
                # sq holds both supertiles: [re2_a | im2_a | re2_b | im2_b]
                sq = sq_pool.tile([MT, nm * 2 * ST], bf16, tag=f"sq{nm}")
                for k, s in enumerate(members):
                    sa, sb, ca, cb = sched[s]
                    ps = ps_pool.tile([MT, 2 * ST], f32, tag="ps")
                    for half, (slot, cblk) in enumerate(((sa, ca), (sb, cb))):
                        l_re = t_c[0:K2, bass.ds(SLOT_COL[slot], MT)]
                        l_im = t_c[K2:128, bass.ds(SLOT_COL[slot], MT)]
                        r_re = t_c[0:K2, bass.ds(RHSB_COL[cblk], NT)]
                        r_im = t_c[K2:128, bass.ds(RHSB_COL[cblk], NT)]
                        nc.tensor.matmul(
                            ps[:, bass.ds(half * NT, NT)], l_re, r_re,
                            start=True, stop=True, tile_position=(0, 0),
                        )
                        nc.tensor.matmul(
                            ps[:, bass.ds(ST + half * NT, NT)], l_im, r_im,
                            start=True, stop=True, tile_position=(64, 0),
                        )
                    # column-split squares for this supertile
                    base = k * 2 * ST
                    nc.scalar.square(
                        sq[:, bass.ds(base + XD, 2 * ST - XD)],
                        ps[:, bass.ds(XD, 2 * ST - XD)],
                    )                                                   # ACT
                    if k == 0:
                        t_cp = tcopy_pool.tile([MT, nm * XD], bf16, tag=f"tcp{nm}")
                    nc.vector.tensor_copy(
                        t_cp[:, bass.ds(k * XD, XD)], ps[:, bass.ds(0, XD)]
                    )                                                   # DVE
                if True:
                    # one packed-bf16 square covering every member's DVE columns
                    sqv = sq[:].rearrange("p (s c) -> p s c", s=2 * nm)
                    nc.vector.tensor_mul(
                        sqv[:, 0:2 * nm:2, 0:XD],
                        t_cp[:].rearrange("p (s c) -> p s c", s=nm),
                        t_cp[:].rearrange("p (s c) -> p s c", s=nm),
                    )                                                   # DVE 2x 2x
                # fused add across the pair: gather [re2_a, re2_b] + [im2_a, im2_b]
                s_t = ssum_pool.tile([MT, nm * ST], bf16, tag=f"ssum{nm}")
                sq3 = sq[:].rearrange("p (s c) -> p s c", s=2 * nm)
                nc.vector.tensor_add(
                    s_t[:].rearrange("p (s c) -> p s c", s=nm),
                    sq3[:, 0:2 * nm:2, :],
                    sq3[:, 1:2 * nm:2, :],
                )                                                       # DVE 2x
                o = opool.tile([MT, nm * ST], f32, tag=f"o{nm}")
                if nm > 1:
                    nc.scalar.sqrt(o[:], s_t[:])                        # ACT
                    nc.sync.dma_start(
                        out=out[bass.ds(members[0] * MT, nm * MT), :]
                            .rearrange("(s p) c -> p s c", s=nm),
                        in_=o[:].rearrange("p (s c) -> p s c", s=nm),
                    )
                else:
                    # final supertile: halve the sqrt so the first half's DMA
                    # overlaps the second sqrt and the kernel-ending transfer
                    # (which exec time waits on) is only 256KB
                    for h in range(2):
                        nc.scalar.sqrt(
                            o[:, bass.ds(h * NT, NT)], s_t[:, bass.ds(h * NT, NT)]
                        )
                        nc.sync.dma_start(
                            out=out[bass.ts(members[0], MT), bass.ds(h * NT, NT)],
                            in_=o[:, bass.ds(h * NT, NT)],
                        )
    nc.finalize()
    return nc


def _woodbury_host(omega, U, d, log_eta):
    """complex128 host algebra. Returns a [n], V [n,r], W [n,r]."""
    U = np.asarray(U, np.float64)
    d = np.asarray(d, np.float64)
    eta = float(np.exp(np.float64(np.asarray(log_eta))))
    z = complex(float(np.asarray(omega)), eta)
    a = 1.0 / (z - d)                      # [n] complex128
    W = a[:, None] * U                     # [n, r]
    B = U.T @ W                            # [r, r]
    M = np.linalg.inv(np.eye(RANK) - B)    # [r, r]
    V = W @ M                              # [n, r]
    return a, V, W


def _core_layout(c):
    """(lhs row slices, rhs col blocks) for core c."""
    # lhs slots 0-3: mc tiles of row block c; slots 4-5: half-block rows
    hb = c % 4
    lhs_rows = [c * BLK + m * MT for m in range(4)]
    off = 0 if c < 4 else 2
    lhs_rows += [hb * BLK + (off + m) * MT for m in range(2)]
    rhs_blocks = [(c + d) % 8 for d in range(4)] + [hb + 4]
    return lhs_rows, rhs_blocks


def _prepare(omega, H_low_rank, H_diag, log_eta):
    """Host Woodbury + per-core input maps. Returns (in_maps, diag_vals)."""
    import ml_dtypes

    a, V, W = _woodbury_host(omega, H_low_rank, H_diag, log_eta)
    Vr = V.real.astype(np.float32); Vi = V.imag.astype(np.float32)
    Wr = W.real.astype(np.float32); Wi = W.imag.astype(np.float32)

    in_maps = []
    for c in range(NCORES):
        consts = np.zeros((128, CONSTS_W), np.float32)
        lhs_rows, rhs_blocks = _core_layout(c)
        for m, r0 in enumerate(lhs_rows):
            cs = slice(SLOT_COL[m], SLOT_COL[m] + MT)
            consts[0:RANK, cs] = Vr[r0:r0 + MT].T
            consts[RANK:K2, cs] = -Vi[r0:r0 + MT].T
            consts[K2:K2 + RANK, cs] = Vr[r0:r0 + MT].T
            consts[K2 + RANK:128, cs] = Vi[r0:r0 + MT].T
        for j, g in enumerate(rhs_blocks):
            cs = slice(RHSB_COL[j], RHSB_COL[j] + NT)
            g0 = g * BLK
            consts[0:RANK, cs] = Wr[g0:g0 + NT].T
            consts[RANK:K2, cs] = Wi[g0:g0 + NT].T
            consts[K2:K2 + RANK, cs] = Wi[g0:g0 + NT].T
            consts[K2 + RANK:128, cs] = Wr[g0:g0 + NT].T
        in_maps.append({"consts": consts.astype(ml_dtypes.bfloat16)})

    diag = a + np.einsum("ij,ij->i", V, W)             # G[i,i] = a_i + (V W^T)[i,i]
    return in_maps, np.minimum(np.abs(diag), CLAMP).astype(np.float32)


def _assemble(results, diag_vals):
    out = np.empty((N, N), np.float32)
    mirrored = []
    for c in range(NCORES):
        R = results[c]["out"]                          # [1152, 1024]
        lhs_rows, rhs_blocks = _core_layout(c)
        for s in range(8):
            mc = SCHED[s][0]
            r0 = c * BLK + mc * MT
            for half in (0, 1):
                g = rhs_blocks[SCHED[s][2 + half]]
                out[r0:r0 + MT, g * BLK:g * BLK + NT] = \
                    R[s * MT:(s + 1) * MT, half * NT:(half + 1) * NT]
        g = rhs_blocks[4]
        for half in (0, 1):
            r0 = lhs_rows[4 + half]
            out[r0:r0 + MT, g * BLK:g * BLK + NT] = \
                R[8 * MT:9 * MT, half * NT:(half + 1) * NT]
        # strict-upper pairs this core computed (for mirroring)
        for d in range(1, 4):
            mirrored.append((c, (c + d) % 8))
        if c < 4:
            mirrored.append((c, c + 4))
    for (i, j) in mirrored:
        out[j * BLK:(j + 1) * BLK, i * BLK:(i + 1) * BLK] = \
            out[i * BLK:(i + 1) * BLK, j * BLK:(j + 1) * BLK].T
    np.fill_diagonal(out, diag_vals)
    return out


def kernel(omega, H_low_rank, H_diag, log_eta):
    from concourse.bass_utils import run_bass_kernel_spmd

    in_maps, diag_vals = _prepare(omega, H_low_rank, H_diag, log_eta)
    if "nc" not in _CACHE:
        _CACHE["nc"] = _build_program()
    res = run_bass_kernel_spmd(_CACHE["nc"], in_maps, list(range(NCORES)))
    return _assemble(res.results, diag_vals)
